# revision 49
# baseline (speedup 1.0000x reference)
"""Trainium2 Bass kernel for a 4-layer Mamba (selective SSM) event-denoising stack.

Model (per reference):
  x = features @ emb_W + emb_b                       [B, L, 128]
  4x mamba layers (d_inner=256, d_state=16, d_conv=4, dt_rank=8)
  out = sigmoid(x @ head_W + head_b)                 [B, L, 1]

Sharding over 8 NeuronCores: data-parallel over batch (4) x tensor-parallel
over d_inner (2).  Core c owns batch c//2 and d_inner half c%2.  Per layer,
two pairwise bf16 AllReduces per quarter: the x-projection partial [40, LQ]
and the out-proj partial [128, LQ]; both triggers are deferred a chunk so
they never head-of-line-block the gpsimd queue while their inputs land.

Per-core dataflow per chunk (T=512), [partition=channel, free=time], as a
3-stage software pipeline (pre2 two chunks ahead, pre1 one chunk ahead):
  pre2: one partition-broadcast DMA writes the 16 B rows into the du tile
        (gpsimd/SWDGE queue, so the scalar/ACT queue is never blocked by
        ring backpressure) + the dt-rank rows.
  pre1: C rows broadcast into the hc tile; dt = softplus(w_dt.T@dtr+dt_b)
        as relu(u) + ln(1+exp(-|u|)) (keeps the ACT Ln-table input in
        (1,2] -- feeding e^u overflows the table's accurate range); dtx;
        du *= dtx in one 2x-mode DVE op; dA[:,s,:] = exp(A_s*dt), 16 ACT
        exps into one [128,16,T] tile.
  post (scan): chunk-boundary carry h_last injected into du column 0, dA
        column 0 zeroed, then ONE fused tensor_tensor_scan over
        [128, 16*T] -- all 16 states in a single DVE instruction (the
        zeroed dA columns reset the internal state at each boundary); h is
        written IN PLACE over dA.
  post (rest): hc = h*C (2x-mode); h_last snapshot; y = sum_s hc_s + xc*Dp
        via 17 identity-matmul PSUM accumulations; y2 = y*silu(z);
        out partial = w_out.T @ y2.
  The emission interleave runs the scan of chunk c-2 before the producers
  of c-1/c so the DVE never head-of-line-blocks on the ACT chain; phase A
  (conv-fused in_proj via 4 PSUM matmuls, z-gate, x-projection) is emitted
  per quarter QLAG=2 quarters ahead, with embeds pulled in lazily.
"""

import sys

sys.path.insert(0, "/opt/trn_rl_repo")

import numpy as np

N_LAYERS = 4
D_MODEL = 128
D_STATE = 16
D_CONV = 4
D_INNER = 256
DT_RANK = 8
FEAT = 16
BATCH = 4
SEQ = 8192

N_CORES = 8
D_OWN = D_INNER // 2  # 128 channels per core
XP = DT_RANK + 2 * D_STATE  # 40

# activation-table ids for gen3: 0=exp_and_others, 5=natural_log,
# 6=natural_log_exp_and_others (has both Exp and Ln)
_ACT_TABLE_MERGE = {0: 6, 5: 6}


def _fix_act_tables(nc):
    """Retarget Exp-only/Ln-only table loads to the table containing both,
    then drop consecutive duplicate loads."""
    import concourse.mybir as mybir

    for b in nc.main_func.blocks:
        keep = []
        cur = None
        for ins in b.instructions:
            if isinstance(ins, mybir.InstLoadActFuncSet):
                tid = _ACT_TABLE_MERGE.get(ins.act_func_set_id, ins.act_func_set_id)
                ins.act_func_set_id = tid
                if tid == cur and not ins.has_wait() and not ins.has_update():
                    continue
                cur = tid
            keep.append(ins)
        if len(keep) != len(b.instructions):
            b.instructions[:] = keep


def build_program(L=SEQ, T=512, no_cc=False, pool_states=0, debug_probes=False,
                  dbg_l=0, debug_dram=False):
    """Build the SPMD Bass/Tile program (same program for all 8 cores)."""
    from contextlib import ExitStack

    import concourse.bass as bass
    import concourse.tile as tile
    from concourse import bacc, mybir

    DT = mybir.dt.float32
    BF = mybir.dt.bfloat16
    F32R = mybir.dt.float32r
    AF = mybir.ActivationFunctionType
    OP = mybir.AluOpType

    NCH = L // T
    NQ = max(2, min(4, NCH // 2))
    while NCH % NQ:
        NQ -= 1
    CPQ = NCH // NQ
    LQ = L // NQ
    QLAG = max(1, NQ // 2)
    HALO = D_CONV - 1
    RING = CPQ * QLAG + 1

    def f32r(ap):
        return ap.bitcast(F32R)

    nc = bacc.Bacc(
        "TRN2",
        target_bir_lowering=False,
        debug=False,
        enable_asserts=False,
        num_devices=N_CORES,
    )

    # ---- external inputs (per-core data; same names on every core) ----
    featT = nc.dram_tensor("featT", [FEAT, L], DT, kind="ExternalInput").ap()
    emb_w = nc.dram_tensor("emb_w", [FEAT, D_MODEL], DT, kind="ExternalInput").ap()
    emb_b = nc.dram_tensor("emb_b", [D_MODEL, 1], DT, kind="ExternalInput").ap()
    head_w = nc.dram_tensor("head_w", [D_MODEL, 1], BF, kind="ExternalInput").ap()
    head_b = nc.dram_tensor("head_b", [1, 1], DT, kind="ExternalInput").ap()
    ident = nc.dram_tensor("ident", [D_OWN, D_OWN], BF, kind="ExternalInput").ap()

    lw = []
    for l in range(N_LAYERS):
        d = dict(
            w_z=nc.dram_tensor(f"w_z_{l}", [D_MODEL, D_OWN], BF, kind="ExternalInput").ap(),
            convb=nc.dram_tensor(f"convb_{l}", [D_OWN, 1], DT, kind="ExternalInput").ap(),
            w_xp=nc.dram_tensor(f"w_xp_{l}", [D_OWN, XP], BF, kind="ExternalInput").ap(),
            w_dt=nc.dram_tensor(f"w_dt_{l}", [DT_RANK, D_OWN], BF, kind="ExternalInput").ap(),
            dt_b=nc.dram_tensor(f"dt_b_{l}", [D_OWN, 1], DT, kind="ExternalInput").ap(),
            a_neg=nc.dram_tensor(f"a_neg_{l}", [D_OWN, D_STATE], DT, kind="ExternalInput").ap(),
            dp=nc.dram_tensor(f"dp_{l}", [D_OWN, 1], DT, kind="ExternalInput").ap(),
            w_out=nc.dram_tensor(f"w_out_{l}", [D_OWN, D_MODEL], BF, kind="ExternalInput").ap(),
        )
        for k in range(D_CONV):
            d[f"w_cxi{k}"] = nc.dram_tensor(f"w_cxi_{l}_{k}", [D_MODEL, D_OWN], BF, kind="ExternalInput").ap()
        lw.append(d)

    probs = nc.dram_tensor("probs", [1, L], DT, kind="ExternalOutput").ap()
    ddbg = {}
    if debug_probes:
        for nm in ["dtc", "dtx", "xc", "zs", "y2", "da0", "da15", "h0", "h15",
                   "du0", "du15", "hc0", "hc15", "xcdp"]:
            ddbg[f"p_{nm}"] = nc.dram_tensor(f"ddbg_p_{nm}", [D_OWN * 512], mybir.dt.bfloat16, kind="ExternalOutput").ap()
    if debug_dram:
        ddbg["x_cur"] = nc.dram_tensor("ddbg_x_cur", [L * D_MODEL], mybir.dt.bfloat16, kind="ExternalOutput").ap()
        for l in range(N_LAYERS):
            ddbg[f"xbc_{l}"] = nc.dram_tensor(f"ddbg_xbc_{l}", [L * XP], mybir.dt.bfloat16, kind="ExternalOutput").ap()
            ddbg[f"xn_{l}"] = nc.dram_tensor(f"ddbg_xn_{l}", [L * D_MODEL], mybir.dt.bfloat16, kind="ExternalOutput").ap()
            ddbg[f"op_{l}"] = nc.dram_tensor(f"ddbg_op_{l}", [L * D_MODEL], mybir.dt.bfloat16, kind="ExternalOutput").ap()

    groups = [[2 * b, 2 * b + 1] for b in range(BATCH)]

    with tile.TileContext(nc) as tc, ExitStack() as ctx:
        wpool = ctx.enter_context(tc.tile_pool(name="w", bufs=1))
        ring_pool = ctx.enter_context(tc.tile_pool(name="ring", bufs=RING))
        cpool = ctx.enter_context(tc.tile_pool(name="chunk", bufs=2))
        dupool = ctx.enter_context(tc.tile_pool(name="dup", bufs=3))
        hcpool = ctx.enter_context(tc.tile_pool(name="hcp", bufs=2))
        dapool = ctx.enter_context(tc.tile_pool(name="da", bufs=2))
        dtpool = ctx.enter_context(tc.tile_pool(name="dtp", bufs=3))
        dt1pool = ctx.enter_context(tc.tile_pool(name="dt1", bufs=1))
        dtps = ctx.enter_context(tc.tile_pool(name="dtps", bufs=1, space="PSUM"))
        qpool = ctx.enter_context(tc.tile_pool(name="q", bufs=2))
        pmm = ctx.enter_context(tc.tile_pool(name="pmm", bufs=4, space="PSUM"))
        ypsum = ctx.enter_context(tc.tile_pool(name="ypsum", bufs=2, space="PSUM"))
        dram = ctx.enter_context(tc.tile_pool(name="dram", bufs=1, space="DRAM"))

        def load_w(ap, shape, tag, dtype=DT):
            t = wpool.tile(shape, dtype, tag=tag)
            if dtype == DT:
                nc.gpsimd.dma_start(f32r(t[:]), f32r(ap))
            else:
                nc.gpsimd.dma_start(t[:], ap)
            return t

        # ---- preload all weights to SBUF ----
        emb_w_sb = load_w(emb_w, [FEAT, D_MODEL], "emb_w")
        emb_b_sb = load_w(emb_b, [D_MODEL, 1], "emb_b")
        head_w_sb = load_w(head_w, [D_MODEL, 1], "head_w", BF)
        head_b_sb = load_w(head_b, [1, 1], "head_b")
        id_sb = load_w(ident, [D_OWN, D_OWN], "ident", BF)
        lsb = []
        for l in range(N_LAYERS):
            d = dict(
                w_z=load_w(lw[l]["w_z"], [D_MODEL, D_OWN], f"w_z{l}", BF),
                convb=load_w(lw[l]["convb"], [D_OWN, 1], f"convb{l}"),
                w_xp=load_w(lw[l]["w_xp"], [D_OWN, XP], f"w_xp{l}", BF),
                w_dt=load_w(lw[l]["w_dt"], [DT_RANK, D_OWN], f"w_dt{l}", BF),
                dt_b=load_w(lw[l]["dt_b"], [D_OWN, 1], f"dt_b{l}"),
                a_neg=load_w(lw[l]["a_neg"], [D_OWN, D_STATE], f"a_neg{l}"),
                dp=load_w(lw[l]["dp"], [D_OWN, 1], f"dp{l}"),
                w_out=load_w(lw[l]["w_out"], [D_OWN, D_MODEL], f"w_out{l}", BF),
            )
            for k in range(D_CONV):
                d[f"w_cxi{k}"] = load_w(lw[l][f"w_cxi{k}"], [D_MODEL, D_OWN], f"w_cxi{l}_{k}", BF)
            lsb.append(d)

        zpad = wpool.tile([D_MODEL, HALO], BF, tag="zpad")
        nc.vector.memset(zpad[:], 0.0)

        # DRAM intermediates (quarter-major so collectives see contiguous blocks)
        x_cur = dram.tile([NQ, D_MODEL, LQ], BF, tag="x0")
        LT = []
        for l in range(N_LAYERS):
            LT.append(dict(
                xdbl_part=dram.tile([NQ, XP, LQ], BF, tag=f"xdblp{l}", name=f"xdblp{l}"),
                xdbl_full=dram.tile([NQ, XP, LQ], BF, tag=f"xdblf{l}", name=f"xdblf{l}"),
                out_part=dram.tile([NQ, D_MODEL, LQ], BF, tag=f"outp{l}", name=f"outp{l}"),
                x_next=dram.tile([NQ, D_MODEL, LQ], BF, tag=f"x{l + 1}", name=f"xn{l + 1}"),
                ring={},       # chunk -> xz ring tile
                pre={},        # chunk -> dict of pre-stage tiles
                h_prev=None,   # previous chunk's h tile
            ))
        x_src = [x_cur] + [LT[l]["x_next"] for l in range(N_LAYERS)]

        # ---- embedding: x0 = emb_W.T @ featT (+ emb_b) ----
        def emit_embed(c):
            sl = slice(c * T, (c + 1) * T)
            q, lc = divmod(c, CPQ)
            f_c = cpool.tile([FEAT, T], DT, tag="f_c")
            nc.sync.dma_start(f32r(f_c[:]), f32r(featT[:, sl]))
            x_ps = pmm.tile([D_MODEL, T], DT, tag="mm")
            nc.tensor.matmul(x_ps[:], f32r(emb_w_sb[:]), f32r(f_c[:]), start=True, stop=True)
            x_sb = cpool.tile([D_MODEL, T], BF, tag="x_sb")
            nc.scalar.activation(x_sb[:], x_ps[:], AF.Identity, bias=emb_b_sb[:, 0:1])
            nc.sync.dma_start(x_cur[q, :, lc * T:(lc + 1) * T], x_sb[:])

        # ---- phase A: in_proj + conv + xproj + z-gate ----
        def emit_A(l, c):
            W = lsb[l]
            xin = x_src[l]
            q, lc = divmod(c, CPQ)
            x_ext = cpool.tile([D_MODEL, T + HALO], BF, tag="x_ext")
            if lc == 0:
                if c == 0:
                    nc.sync.dma_start(x_ext[:, 0:HALO], zpad[:])
                else:
                    nc.sync.dma_start(x_ext[:, 0:HALO], xin[q - 1, :, LQ - HALO:LQ])
                nc.sync.dma_start(x_ext[:, HALO:], xin[q, :, 0:T])
            else:
                nc.sync.dma_start(x_ext[:], xin[q, :, lc * T - HALO:(lc + 1) * T])

            ring = ring_pool.tile([D_OWN, 2, T], BF, tag=f"xz{l % 2}")
            LT[l]["ring"][c] = ring

            # conv(in_proj(x)) = sum_k (in_W_xi * conv_w[:,k]).T @ x[t-3+k]
            xc_ps = pmm.tile([D_OWN, T], DT, tag="mm")
            for k in range(D_CONV):
                nc.tensor.matmul(
                    xc_ps[:], W[f"w_cxi{k}"][:], x_ext[:, k:k + T],
                    start=(k == 0), stop=(k == D_CONV - 1),
                )
            nc.scalar.activation(ring[:, 0, :], xc_ps[:], AF.Silu, bias=W["convb"][:, 0:1])

            # xproj partial: [40, T] = w_xp.T @ xc  (host-ordered [B, C, dt])
            xp_ps = pmm.tile([XP, T], DT, tag="mm")
            nc.tensor.matmul(xp_ps[:], W["w_xp"][:], ring[:, 0, :], start=True, stop=True)
            xp_sb = cpool.tile([XP, T], BF, tag="xp_sb")
            nc.scalar.activation(xp_sb[:], xp_ps[:], AF.Copy)
            nc.sync.dma_start(LT[l]["xdbl_part"][q, :, lc * T:(lc + 1) * T], xp_sb[:])

            # z-gate: zs = silu(x @ w_z)
            z_ps = pmm.tile([D_OWN, T], DT, tag="mm")
            nc.tensor.matmul(z_ps[:], W["w_z"][:], x_ext[:, HALO:], start=True, stop=True)
            nc.scalar.activation(ring[:, 1, :], z_ps[:], AF.Silu)

        # ---- ARx (single bf16 collective; rows [B, C, dt]) ----
        def emit_ARx(l, q):
            if no_cc:
                nc.sync.dma_start(LT[l]["xdbl_full"][q], LT[l]["xdbl_part"][q])
            else:
                nc.gpsimd.collective_compute(
                    "AllReduce", mybir.AluOpType.add, replica_groups=groups,
                    ins=[LT[l]["xdbl_part"][q].opt()],
                    outs=[LT[l]["xdbl_full"][q].opt()],
                )

        pending_arx = []

        def A_quarter(l, q):
            for lc in range(CPQ):
                emit_A(l, q * CPQ + lc)
            pending_arx.append((l, q))

        # ---- phase B pre2 (2 chunks ahead): B-broadcast + dt-rank rows ----
        def emit_B_pre2(l, c):
            while pending_arx:
                emit_ARx(*pending_arx.pop(0))
            W = lsb[l]
            q, lc = divmod(c, CPQ)
            xdf = LT[l]["xdbl_full"]
            csl = slice(lc * T, (lc + 1) * T)
            du = dupool.tile([D_OWN, D_STATE, T], BF, tag="du")
            nc.gpsimd.dma_start(du[:], xdf[q, 0:D_STATE, csl].partition_broadcast(D_OWN))
            dtr_c = dtpool.tile([DT_RANK, T], BF, tag="dtr")
            nc.sync.dma_start(dtr_c[:], xdf[q, 2 * D_STATE:XP, csl])

            LT[l]["pre"][c] = dict(du=du, dtr=dtr_c)

        # ---- phase B pre1 (1 chunk ahead): C-broadcast + dt/dA producers ----
        def emit_B_pre1(l, c):
            W = lsb[l]
            q, lc = divmod(c, CPQ)
            xdf = LT[l]["xdbl_full"]
            csl = slice(lc * T, (lc + 1) * T)
            pre = LT[l]["pre"][c]
            xc = LT[l]["ring"][c][:, 0, :]

            hc = hcpool.tile([D_OWN, D_STATE, T], BF, tag="hc")
            nc.gpsimd.dma_start(hc[:], xdf[q, D_STATE:2 * D_STATE, csl].partition_broadcast(D_OWN))
            pre["hc"] = hc

            # dt = softplus(u) = relu(u) + ln(1 + exp(-|u|)), u = w_dt.T@dtr + dt_b
            # (keeps the Ln table input in (1,2]; all four funcs live in table 6)
            dt_ps = pmm.tile([D_OWN, T], DT, tag="mm")
            nc.tensor.matmul(dt_ps[:], W["w_dt"][:], pre.pop("dtr")[:], start=True, stop=True)
            dt_a = dtps.tile([D_OWN, T], DT, tag="dt_a")
            nc.scalar.activation(dt_a[:], dt_ps[:], AF.Abs, bias=W["dt_b"][:, 0:1])
            dt_r = dt1pool.tile([D_OWN, T], DT, tag="dt_r")
            nc.scalar.activation(dt_r[:], dt_ps[:], AF.Relu, bias=W["dt_b"][:, 0:1])
            dt_e = dtps.tile([D_OWN, T], DT, tag="dt_e")
            nc.scalar.activation(dt_e[:], dt_a[:], AF.Exp, scale=-1.0)
            dt_l = dt1pool.tile([D_OWN, T], DT, tag="dt_l")
            nc.scalar.activation(dt_l[:], dt_e[:], AF.Ln, bias=1.0)
            dtc = dtpool.tile([D_OWN, T], BF, tag="dtc")
            nc.vector.tensor_add(dtc[:], dt_r[:], dt_l[:])

            # dtx = dt * xc
            dtx = dtpool.tile([D_OWN, T], BF, tag="dtx")
            nc.vector.tensor_mul(dtx[:], dtc[:], xc)
            pre["dtx"] = dtx

            # du = B * dtx (all 16 states, one 2x-mode op)
            dtx_bc = pre["dtx"][:].unsqueeze(1).broadcast_to((D_OWN, D_STATE, T))
            nc.vector.tensor_tensor(pre["du"][:], pre["du"][:], dtx_bc, op=OP.mult)

            # dA[:, s, :] = exp(A_s * dt)
            dA = dapool.tile([D_OWN, D_STATE, T], BF, tag="dA")
            for s in range(D_STATE):
                nc.scalar.activation(dA[:, s, :], dtc[:], AF.Exp,
                                     scale=W["a_neg"][:, s:s + 1])
            if c == 0:
                nc.vector.memset(dA[:, :, 0:1], 0.0)
            pre["dA"] = dA

        # ---- phase B post (scan part): carry + fused scan ----
        def emit_B_post_scan(l, c):
            pre = LT[l]["pre"][c]
            du, dA = pre["du"], pre["dA"]

            # chunk-boundary carry from the h_last snapshot:
            # du[:, s, 0] += dA[:, s, 0] * h_prev_last[:, s], then zero dA
            # column 0 so the fused scan resets at each state boundary.
            if c > 0:
                carry = qpool.tile([D_OWN, D_STATE, 1], BF, tag="carry")
                nc.vector.tensor_tensor(carry[:], dA[:, :, 0:1],
                                        LT[l]["h_last"][:], op=OP.mult)
                nc.vector.tensor_tensor(du[:, :, 0:1], du[:, :, 0:1],
                                        carry[:], op=OP.add)
                nc.vector.memset(dA[:, :, 0:1], 0.0)

            # ONE fused scan over all 16 states [128, 16*T]; h written IN PLACE
            # over dA (the a-operand is consumed before each output element).
            def flat2d(ap):
                return bass.AP(tensor=ap.tensor, offset=ap.offset,
                               ap=[list(ap.ap[0]), [1, D_STATE * T]])

            nc.vector.tensor_tensor_scan(flat2d(dA[:]), flat2d(dA[:]),
                                         flat2d(du[:]), 0.0,
                                         op0=OP.mult, op1=OP.add)

        # ---- phase B post (rest): gate + reduction + out projection ----
        def emit_B_post_rest(l, c):
            W = lsb[l]
            q, lc = divmod(c, CPQ)
            pre = LT[l]["pre"].pop(c)
            h, hc = pre["dA"], pre["hc"]
            ring = LT[l]["ring"].pop(c)
            xc = ring[:, 0, :]
            zs = ring[:, 1, :]

            # hc = h * C in place (one 2x-mode op)
            nc.vector.tensor_tensor(hc[:], hc[:], h[:], op=OP.mult)

            # snapshot the chunk-final states for the next chunk's carry
            h_last = qpool.tile([D_OWN, D_STATE, 1], BF, tag="hlast")
            nc.vector.tensor_copy(h_last[:], h[:, :, T - 1:T])
            LT[l]["h_last"] = h_last

            # y = sum_s hc_s + xc*Dp via identity-matmul PSUM accumulation
            xcdp = cpool.tile([D_OWN, T], BF, tag="xcdp")
            nc.vector.tensor_scalar_mul(xcdp[:], xc, W["dp"][:, 0:1])
            y_ps = ypsum.tile([D_OWN, T], DT, tag="y")
            nc.tensor.matmul(y_ps[:], id_sb[:], xcdp[:], start=True, stop=False)
            for s in range(D_STATE):
                nc.tensor.matmul(y_ps[:], id_sb[:], hc[:, s, :],
                                 start=False, stop=(s == D_STATE - 1))

            # y2 = y * silu(z); out partial = w_out.T @ y2
            y2 = cpool.tile([D_OWN, T], BF, tag="y2")
            nc.vector.tensor_mul(y2[:], y_ps[:], zs)
            o_ps = pmm.tile([D_MODEL, T], DT, tag="mm")
            nc.tensor.matmul(o_ps[:], W["w_out"][:], y2[:], start=True, stop=True)
            o_sb = cpool.tile([D_MODEL, T], BF, tag="o_sb")
            nc.scalar.activation(o_sb[:], o_ps[:], AF.Copy)
            nc.sync.dma_start(LT[l]["out_part"][q, :, lc * T:(lc + 1) * T], o_sb[:])

        def emit_ARout(l, q):
            if no_cc:
                nc.sync.dma_start(LT[l]["x_next"][q], LT[l]["out_part"][q])
            else:
                nc.gpsimd.collective_compute(
                    "AllReduce", mybir.AluOpType.add, replica_groups=groups,
                    ins=[LT[l]["out_part"][q].opt()],
                    outs=[LT[l]["x_next"][q].opt()],
                )

        # ---- head: logits -> sigmoid fused, straight to probs ----
        def emit_head(c):
            q, lc = divmod(c, CPQ)
            x_c = cpool.tile([D_MODEL, T], BF, tag="x_hd")
            nc.sync.dma_start(x_c[:], x_src[N_LAYERS][q, :, lc * T:(lc + 1) * T])
            h_ps = pmm.tile([1, T], DT, tag="mm")
            nc.tensor.matmul(h_ps[:], head_w_sb[:], x_c[:], start=True, stop=True)
            pr = cpool.tile([1, T], DT, tag="fs_out")
            nc.scalar.activation(pr[:], h_ps[:], AF.Sigmoid, bias=head_b_sb[:, 0:1])
            nc.sync.dma_start(probs[:, c * T:(c + 1) * T], pr[:])

        def head_quarter(hq):
            for lc in range(CPQ):
                emit_head(hq * CPQ + lc)

        # ---- schedule ----
        emitted_A = set()
        embedded_q = set()

        def need_A(l2, q2):
            if q2 >= NQ or (l2, q2) in emitted_A:
                return
            emitted_A.add((l2, q2))
            if l2 == 0 and q2 not in embedded_q:
                embedded_q.add(q2)
                for lc in range(CPQ):
                    emit_embed(q2 * CPQ + lc)
            if l2 == 0 and q2 + 1 < NQ and q2 + 1 not in embedded_q:
                # keep the embed of the next quarter one quarter ahead of
                # its phase A (the conv halo reads across the boundary).
                embedded_q.add(q2 + 1)
                for lc in range(CPQ):
                    emit_embed((q2 + 1) * CPQ + lc)
            A_quarter(l2, q2)

        for q0 in range(QLAG):
            need_A(0, q0)

        # global chunk stream with a 1-chunk pre/post software lag.
        seq = [(l, c) for l in range(N_LAYERS) for c in range(NCH)]

        def post_actions(l, c):
            """Quarter-boundary actions after emit_B_post(l, c)."""
            while pending_arx:
                emit_ARx(*pending_arx.pop(0))
            if c % CPQ != CPQ - 1:
                return
            q = c // CPQ
            emit_ARout(l, q)
            nq = q + QLAG
            if nq < NQ:
                need_A(l, nq)
            elif l < N_LAYERS - 1:
                need_A(l + 1, nq - NQ)
            else:
                hq = q - QLAG
                if hq >= 0:
                    head_quarter(hq)

        drained = 0
        pre1_done = 0

        def ensure_pre1(upto):
            nonlocal pre1_done
            while pre1_done < upto:
                emit_B_pre1(*seq[pre1_done])
                pre1_done += 1

        def ensure_post(upto):
            nonlocal drained
            while drained < upto:
                ensure_pre1(drained + 1)
                pl, pc = seq[drained]
                emit_B_post_scan(pl, pc)
                emit_B_post_rest(pl, pc)
                post_actions(pl, pc)
                drained += 1

        for i, (l, c) in enumerate(seq):
            # Tile dependencies follow emission order: before phase A of a
            # quarter whose AR inputs come from still-pending posts, drain.
            if (l, c // CPQ) not in emitted_A:
                ensure_pre1(i)
                ensure_post(i)
                need_A(l, c // CPQ)
            emit_B_pre2(l, c)
            if i >= 2 and drained == i - 2:
                # steady state: scan(c-2) first (inputs ready), then the
                # pre1 producers of c-1 (they overlap the scan), then the
                # rest of post(c-2).
                pl, pc = seq[i - 2]
                emit_B_post_scan(pl, pc)
                ensure_pre1(i)
                emit_B_post_rest(pl, pc)
                post_actions(pl, pc)
                drained = i - 1
            else:
                ensure_pre1(i)
                ensure_post(i - 1)
        ensure_pre1(len(seq))
        ensure_post(len(seq))
        for hq in range(NQ - QLAG, NQ):
            head_quarter(hq)
        if debug_dram:
            def dcopy(dst, t, n):
                nc.sync.dma_start(
                    bass.AP(tensor=dst.tensor, offset=dst.offset, ap=[[1, n]]),
                    bass.AP(tensor=t.tensor, offset=t.offset, ap=[[1, n]]))
            dcopy(ddbg["x_cur"], x_cur, L * D_MODEL)
            for l in range(N_LAYERS):
                dcopy(ddbg[f"xbc_{l}"], LT[l]["xdbl_full"], L * XP)
                dcopy(ddbg[f"xn_{l}"], LT[l]["x_next"], L * D_MODEL)
                dcopy(ddbg[f"op_{l}"], LT[l]["out_part"], L * D_MODEL)

    nc.compile()
    _fix_act_tables(nc)
    return nc


def _bf16():
    import ml_dtypes

    return ml_dtypes.bfloat16


def make_in_maps(inputs, L=SEQ):
    """Host-side sharding: slice the full inputs into 8 per-core input maps."""
    f32 = np.float32
    bf16 = _bf16()
    features = np.asarray(inputs["features"], f32)
    emb_W = np.asarray(inputs["emb_W"], f32)
    emb_b = np.asarray(inputs["emb_b"], f32)
    in_W = np.asarray(inputs["in_W"], f32)
    conv_w = np.asarray(inputs["conv_w"], f32)
    conv_b = np.asarray(inputs["conv_b"], f32)
    xproj_W = np.asarray(inputs["xproj_W"], f32)
    dt_W = np.asarray(inputs["dt_W"], f32)
    dt_b = np.asarray(inputs["dt_b"], f32)
    A_log = np.asarray(inputs["A_log"], f32)
    Dp = np.asarray(inputs["Dp"], f32)
    out_W = np.asarray(inputs["out_W"], f32)
    head_W = np.asarray(inputs["head_W"], f32)
    head_b = np.asarray(inputs["head_b"], f32)

    a_neg = -np.exp(A_log)

    in_maps = []
    for core in range(N_CORES):
        b, h = core // 2, core % 2
        dsl = slice(h * D_OWN, (h + 1) * D_OWN)
        m = {
            "featT": np.ascontiguousarray(features[b, :L].T),
            "emb_w": emb_W,
            "emb_b": emb_b.reshape(D_MODEL, 1),
            "head_w": head_W.astype(bf16),
            "head_b": head_b.reshape(1, 1),
            "ident": np.eye(D_OWN, dtype=f32).astype(bf16),
        }
        for l in range(N_LAYERS):
            w_xi = in_W[l][:, dsl]
            cw = conv_w[l][dsl]
            for k in range(D_CONV):
                m[f"w_cxi_{l}_{k}"] = np.ascontiguousarray(w_xi * cw[:, k][None, :]).astype(bf16)
            m[f"w_z_{l}"] = np.ascontiguousarray(in_W[l][:, D_INNER:][:, dsl]).astype(bf16)
            m[f"convb_{l}"] = np.ascontiguousarray(conv_b[l][dsl].reshape(D_OWN, 1))
            w_xp_l = xproj_W[l][dsl]
            m[f"w_xp_{l}"] = np.ascontiguousarray(
                np.concatenate([w_xp_l[:, DT_RANK:], w_xp_l[:, :DT_RANK]], axis=1)
            ).astype(bf16)
            m[f"w_dt_{l}"] = np.ascontiguousarray(dt_W[l][:, dsl]).astype(bf16)
            m[f"dt_b_{l}"] = np.ascontiguousarray(dt_b[l][dsl].reshape(D_OWN, 1))
            m[f"a_neg_{l}"] = np.ascontiguousarray(a_neg[l][dsl])
            m[f"dp_{l}"] = np.ascontiguousarray(Dp[l][dsl].reshape(D_OWN, 1))
            m[f"w_out_{l}"] = np.ascontiguousarray(out_W[l][dsl]).astype(bf16)
        in_maps.append(m)
    return in_maps


_CACHE = {}


def _get_program(L=SEQ, T=512):
    key = (L, T)
    if key not in _CACHE:
        _CACHE[key] = build_program(L, T)
    return _CACHE[key]


def run(inputs, L=SEQ, T=512, trace=False):
    from concourse.bass_utils import run_bass_kernel_spmd

    nc = _get_program(L, T)
    in_maps = make_in_maps(inputs, L)
    res = run_bass_kernel_spmd(nc, in_maps, list(range(N_CORES)), trace=trace)
    outs = np.stack(
        [res.results[2 * b]["probs"].reshape(L, 1) for b in range(BATCH)]
    )
    return outs.astype(np.float32), res


def kernel(**inputs) -> np.ndarray:
    out, _ = run(inputs)
    return out
